# revision 32
# baseline (speedup 1.0000x reference)
"""Trainium2 Bass kernel for SimCLR-style contrastive loss (B=8192, D=512).

Math (matches reference):
    f_norm = f / ||f||
    sim    = f_norm @ f_norm.T / T
    lse_i  = logsumexp_{j != i} sim_ij
    pos_i  = sim[i, (i + B/2) mod B]
    loss   = mean_i(lse_i - pos_i)

Device strategy (8 cores, data parallel over rows):
  - Host passes each core a row-rotated, transposed bf16 copy of
    features [D, B] so one SPMD program works for every core: core c's
    rotated copy starts at global row c*R, hence its own rows occupy
    local columns [0, R) and row r's positive partner sits at local
    column r + B/2 (never wraps for r < R <= B/2).
  - Loads split across the sync HWDGE queue (k-tiles 0-1) and the
    gpsimd SWDGE queue (k-tiles 2-3), piece-major so early chunks land
    first; the ACT engine issues no DMA.
  - Normalization: squares on DVE (bf16, 2x mode), column sum-of-
    squares via ones-vector bf16 matmul on PE, then per chunk
    ln -> exp(-0.5*ln + ln 4) on ACT giving 4/norm (the x4 per side
    keeps the fp8 features clear of the e4m3 subnormal range; the x16
    on the gram is compensated in the final exp scale), and
    partition_broadcast on GpSimd to a [128, B] bf16 tile.
  - The normalize multiply runs on DVE with direct fp8e4 output into
    DoubleRow layout [128, 2, B] (k-tile pairs on dim 1).
  - Main gram matmul in fp8e4 with MatmulPerfMode.DoubleRow: each
    instruction contracts K=256 (two k-planes), so D=512 streams each
    output column only twice -- half the bf16 PE cost.  PSUM groups
    are 1536 wide (3 banks x 2 bufs + 2 ssq banks = 8) to amortize
    ACT per-instruction cost.
  - Each [128 x W] PSUM group gets: diagonal killed with a -1e30*I
    add (accumulating bf16 PE matmul), partner diagonal extracted
    with tensor_mul against (-1/(16T))*I + row reduce, then fused
    exp+row-sum on ACT (exp((16cos)/(16T) - 1/T), in place in PSUM).
  - Per-row output y_i = ln(sum_i) - pos_cos_i/T returned as [R] f32;
    host computes loss = 1/T + mean(y).
"""

import functools
import sys

sys.path.insert(0, "/opt/trn_rl_repo")

import ml_dtypes
import numpy as np

import concourse.bass as bass
import concourse.mybir as mybir
import concourse.tile as tile
from concourse import bacc
from concourse.bass_utils import run_bass_kernel_spmd
from concourse.tile import add_dep_helper

B = 8192
D = 512
NCORES = 8
R = B // NCORES  # rows per core
TEMP = 0.07
INV_T = 1.0 / TEMP
FSCALE = 16.0  # features carry 4/norm each -> gram = 16*cos
LN_FS_HALF = float(np.log(4.0))

F32 = mybir.dt.float32
BF16 = mybir.dt.bfloat16
FP8 = mybir.dt.float8e4
AF = mybir.ActivationFunctionType
ALU = mybir.AluOpType
DR = mybir.MatmulPerfMode.DoubleRow


@functools.lru_cache(maxsize=None)
def build(b=B, d=D, r=R):
    """Build the single-core SPMD program (identical on all cores)."""
    KT = d // 128  # k-tiles over feature dim
    QT = KT // 2  # DoubleRow k-pair tiles
    RT = r // 128  # row tiles per core
    CH = 512  # normalization chunk width
    NCH = b // CH
    LW = min(2048, b)  # DMA piece width
    NLW = b // LW
    GWMAX = 1536 if b >= 1536 else b
    gstart, gwidth = [], []
    pos = 0
    while pos < b:
        w = min(GWMAX, b - pos)
        gstart.append(pos)
        gwidth.append(w)
        pos += w
    NG = len(gstart)

    def group_chunks(g):
        if g >= NG:
            return []
        lo = gstart[g] // CH
        hi = (gstart[g] + gwidth[g]) // CH
        return list(range(lo, hi))

    nc = bacc.Bacc(None, target_bir_lowering=False)
    ftd = nc.dram_tensor("features_t", [d, b], BF16, kind="ExternalInput")
    outd = nc.dram_tensor("losses", [r], F32, kind="ExternalOutput")

    with tile.TileContext(nc) as tc:
        with (
            tc.tile_pool(name="ftp", bufs=1) as ftp,
            tc.tile_pool(name="sing", bufs=1) as sing,
            tc.tile_pool(name="sq", bufs=6) as sqp,
            tc.tile_pool(name="nrm", bufs=3) as nrmp,
            tc.tile_pool(name="dump", bufs=2) as dumpp,
            tc.tile_pool(name="ssq", bufs=2, space="PSUM") as ssqp,
            tc.tile_pool(name="mm", bufs=2, space="PSUM") as mmp,
        ):
            ft = [
                ftp.tile([128, b], BF16, tag=f"ft{k}", name=f"ft{k}")
                for k in range(KT)
            ]
            # fp8 normalized features in DoubleRow layout: dim1 = which
            # k-plane of the pair, so [:, :, cols] slices feed matmul
            # directly as [128, 2, N] APs.
            ft8 = [
                ftp.tile([128, 2, b], FP8, tag=f"ft8_{q}", name=f"ft8_{q}")
                for q in range(QT)
            ]

            # Pre-place the one ACT table set that covers every function
            # used below (exp, ln, copy): natural_log_exp_and_others.
            _ln_exp_set = 6  # index in act_info.json act_func_sets
            _tl = mybir.InstLoadActFuncSet(
                name=nc.get_next_instruction_name(),
                act_func_set_id=_ln_exp_set,
                ins=[],
                outs=[],
            )
            nc.scalar.add_instruction(_tl)

            # ---- loads: sync HWDGE + gpsimd SWDGE, k-tiles 0-1 on sync,
            # 2-3 on gpsimd, piece-major so early chunks land first ----
            for p in range(NLW):
                ls = slice(p * LW, (p + 1) * LW)
                for k in range(KT):
                    eng = nc.sync if k < KT // 2 else nc.gpsimd
                    eng.dma_start(
                        out=ft[k][:, ls],
                        in_=ftd[k * 128 : (k + 1) * 128, ls],
                    )

            # Constant tiles.  negI/eye in bf16 so the diagonal poison can
            # be applied by the PE itself (an accumulating I.T @ (-1e30*I)
            # matmul) instead of a DVE pass over PSUM.
            negI = sing.tile([128, 128], BF16)
            nc.gpsimd.memset(negI[:], 0.0)
            nc.gpsimd.affine_select(
                out=negI[:],
                in_=negI[:],
                compare_op=ALU.not_equal,
                fill=-1e30,
                base=0,
                pattern=[[-1, 128]],
                channel_multiplier=1,
            )
            eyeb = sing.tile([128, 128], BF16)
            nc.gpsimd.memset(eyeb[:], 0.0)
            nc.gpsimd.affine_select(
                out=eyeb[:],
                in_=eyeb[:],
                compare_op=ALU.not_equal,
                fill=1.0,
                base=0,
                pattern=[[-1, 128]],
                channel_multiplier=1,
            )
            # (-1/(FSCALE*T)) * I -- multiplying the partner block by this
            # and row-reducing yields -pos_logit directly.
            negTI = sing.tile([128, 128], F32)
            nc.gpsimd.memset(negTI[:], 0.0)
            nc.gpsimd.affine_select(
                out=negTI[:],
                in_=negTI[:],
                compare_op=ALU.not_equal,
                fill=-INV_T / FSCALE,
                base=0,
                pattern=[[-1, 128]],
                channel_multiplier=1,
            )
            ones_k = sing.tile([128, 1], BF16)
            nc.vector.memset(ones_k[:], 1.0)
            negC = sing.tile([128, 1], F32)
            nc.vector.memset(negC[:], -INV_T)
            ln4c = sing.tile([1, 1], F32)
            nc.vector.memset(ln4c[:], LN_FS_HALF)

            inv_row = sing.tile([1, b], BF16)  # 4/norm per column
            bc_sb = sing.tile([128, b], BF16)  # broadcast 4/norm
            spart = sing.tile([128, RT, NG], F32)
            posn = sing.tile([128, RT], F32)
            ysb = sing.tile([128, RT], F32)
            ssum = sing.tile([128, RT], F32)
            lnS = sing.tile([128, RT], F32)

            # ---- normalization, split so the sum-of-squares half (which
            # gates PE/ACT progress) runs well ahead of the fp8 scale
            # half (needed only one group before use). ----
            def sumsq_chunk(ch):
                cs = slice(ch * CH, (ch + 1) * CH)
                ssq = ssqp.tile([1, CH], F32, name=f"ssq{ch}", tag="ssq")
                for k in range(KT):
                    sq = sqp.tile([128, CH], BF16, name=f"sq{ch}_{k}", tag="sq")
                    nc.vector.tensor_mul(sq[:], ft[k][:, cs], ft[k][:, cs])
                    nc.tensor.matmul(
                        ssq[:],
                        ones_k[:],
                        sq[:],
                        start=(k == 0),
                        stop=(k == KT - 1),
                    )
                lns = nrmp.tile([1, CH], F32, name=f"lns{ch}", tag="lns")
                nc.scalar.activation(out=lns[:], in_=ssq[:], func=AF.Ln)
                nc.scalar.activation(
                    out=inv_row[0:1, cs],
                    in_=lns[:],
                    func=AF.Exp,
                    scale=-0.5,
                    bias=ln4c[:],
                )
                nc.gpsimd.partition_broadcast(bc_sb[:, cs], inv_row[0:1, cs])

            def scale_chunk(ch):
                cs = slice(ch * CH, (ch + 1) * CH)
                for k in range(KT):
                    nc.vector.tensor_mul(ft[k][:, cs], ft[k][:, cs], bc_sb[:, cs])

            def cast_group(g):
                """SWDGE casting DMA bf16 -> fp8 for the whole group span
                (the gpsimd queue is the only one whose DMAs cast)."""
                if g >= NG:
                    return
                cs = slice(gstart[g], gstart[g] + gwidth[g])
                for k in range(KT):
                    nc.gpsimd.dma_start(
                        out=ft8[k // 2][:, k % 2, cs], in_=ft[k][:, cs]
                    )

            for ch in group_chunks(0) + group_chunks(1):
                sumsq_chunk(ch)
            for ch in group_chunks(0):
                scale_chunk(ch)
            cast_group(0)
            for g in range(NG):
                g0, gw = gstart[g], gwidth[g]
                for ch in group_chunks(g + 1):
                    scale_chunk(ch)
                cast_group(g + 1)
                for t in range(RT):
                    ps = mmp.tile([128, GWMAX], F32, tag="mm")
                    for n2 in range(gw // 512):
                        for q in range(QT):
                            nc.tensor.matmul(
                                ps[:, n2 * 512 : (n2 + 1) * 512],
                                ft8[q][:, :, t * 128 : (t + 1) * 128],
                                ft8[q][:, :, g0 + n2 * 512 : g0 + (n2 + 1) * 512],
                                start=(q == 0),
                                stop=(q == QT - 1),
                                perf_mode=DR,
                            )
                    # Positive-pair diagonal: global col b/2 + t*128.
                    pcol = b // 2 + t * 128
                    if g0 <= pcol < g0 + gw:
                        off = pcol - g0
                        dmp = dumpp.tile([128, 128], F32)
                        nc.vector.tensor_mul(dmp[:], ps[:, off : off + 128], negTI[:])
                        nc.vector.tensor_reduce(
                            out=posn[:, t : t + 1],
                            in_=dmp[:],
                            axis=mybir.AxisListType.X,
                            op=ALU.add,
                        )
                    # Self-similarity diagonal: global col t*128 -> -1e30,
                    # applied on the PE as one extra accumulating matmul.
                    dcol = t * 128
                    if g0 <= dcol < g0 + gw:
                        off = dcol - g0
                        nc.tensor.matmul(
                            ps[:, off : off + 128],
                            eyeb[:],
                            negI[:],
                            start=False,
                            stop=True,
                            skip_group_check=True,
                        )
                    # exp((16cos - 16)/(16T)) in place + fused row-sum.
                    nc.scalar.activation(
                        out=ps[:, :gw],
                        in_=ps[:, :gw],
                        func=AF.Exp,
                        scale=INV_T / FSCALE,
                        bias=negC[:],
                        accum_out=spart[:, t, g : g + 1],
                    )
                # Lookahead sum-of-squares AFTER this group's matmuls, so
                # PE's in-order queue never gates a group on a later DMA.
                for ch in group_chunks(g + 2):
                    sumsq_chunk(ch)

            # ---- epilogue: y = ln(S) - pos/T ----
            for t in range(RT):
                nc.vector.tensor_reduce(
                    out=ssum[:, t : t + 1],
                    in_=spart[:, t, :],
                    axis=mybir.AxisListType.X,
                    op=ALU.add,
                )
            nc.scalar.activation(out=lnS[:, :RT], in_=ssum[:, :RT], func=AF.Ln)
            nc.vector.tensor_add(ysb[:, :RT], lnS[:, :RT], posn[:, :RT])
            nc.sync.dma_start(
                out=outd[:].rearrange("(t p) -> p t", p=128), in_=ysb[:, :RT]
            )

    nc.finalize()
    return nc


def run(features, b=B, d=D, ncores=NCORES, **kwargs):
    """Run the SPMD kernel; returns (losses[b] fp32, BassKernelResults)."""
    r = b // ncores
    nc = build(b, d, r)
    feats = np.ascontiguousarray(np.asarray(features, dtype=np.float32))
    in_maps = []
    for c in range(ncores):
        rot = np.roll(feats, -c * r, axis=0)
        in_maps.append(
            {"features_t": np.ascontiguousarray(rot.T).astype(ml_dtypes.bfloat16)}
        )
    res = run_bass_kernel_spmd(nc, in_maps, core_ids=list(range(ncores)), **kwargs)
    y = np.concatenate([res.results[c]["losses"] for c in range(ncores)])
    return y, res


def kernel(features):
    y, _ = run(features)
    loss = INV_T + float(np.mean(y.astype(np.float64)))
    return np.float32(loss)


# revision 33
# speedup vs baseline: 1.0205x; 1.0205x over previous
"""Trainium2 Bass kernel for SimCLR-style contrastive loss (B=8192, D=512).

Math (matches reference):
    f_norm = f / ||f||
    sim    = f_norm @ f_norm.T / T
    lse_i  = logsumexp_{j != i} sim_ij
    pos_i  = sim[i, (i + B/2) mod B]
    loss   = mean_i(lse_i - pos_i)

Device strategy (8 cores, data parallel over rows):
  - Host passes each core a row-rotated, transposed bf16 copy of
    features [D, B] so one SPMD program works for every core: core c's
    rotated copy starts at global row c*R, hence its own rows occupy
    local columns [0, R) and row r's positive partner sits at local
    column r + B/2 (never wraps for r < R <= B/2).
  - Loads split across the sync HWDGE queue (k-tiles 0-1) and the
    gpsimd SWDGE queue (k-tiles 2-3), piece-major so early chunks land
    first; the ACT engine issues no DMA.
  - Normalization: squares on DVE (bf16, 2x mode), column sum-of-
    squares via ones-vector bf16 matmul on PE, then per chunk
    ln -> exp(-0.5*ln + ln 4) on ACT giving 4/norm (the x4 per side
    keeps the fp8 features clear of the e4m3 subnormal range; the x16
    on the gram is compensated in the final exp scale), and
    partition_broadcast on GpSimd to a [128, B] bf16 tile.
  - The normalize multiply runs on DVE with direct fp8e4 output into
    DoubleRow layout [128, 2, B] (k-tile pairs on dim 1).
  - Main gram matmul in fp8e4 with MatmulPerfMode.DoubleRow: each
    instruction contracts K=256 (two k-planes), so D=512 streams each
    output column only twice -- half the bf16 PE cost.  PSUM groups
    are 1536 wide (3 banks x 2 bufs + 2 ssq banks = 8) to amortize
    ACT per-instruction cost.
  - Each [128 x W] PSUM group gets: diagonal killed with a -1e30*I
    add (accumulating bf16 PE matmul), partner diagonal extracted
    with tensor_mul against (-1/(16T))*I + row reduce, then fused
    exp+row-sum on ACT (exp((16cos)/(16T) - 1/T), in place in PSUM).
  - Per-row output y_i = ln(sum_i) - pos_cos_i/T returned as [R] f32;
    host computes loss = 1/T + mean(y).
"""

import functools
import sys

sys.path.insert(0, "/opt/trn_rl_repo")

import ml_dtypes
import numpy as np

import concourse.bass as bass
import concourse.mybir as mybir
import concourse.tile as tile
from concourse import bacc
from concourse.bass_utils import run_bass_kernel_spmd
from concourse.tile import add_dep_helper

B = 8192
D = 512
NCORES = 8
R = B // NCORES  # rows per core
TEMP = 0.07
INV_T = 1.0 / TEMP
FSCALE = 16.0  # features carry 4/norm each -> gram = 16*cos
LN_FS_HALF = float(np.log(4.0))

F32 = mybir.dt.float32
BF16 = mybir.dt.bfloat16
FP8 = mybir.dt.float8e4
AF = mybir.ActivationFunctionType
ALU = mybir.AluOpType
DR = mybir.MatmulPerfMode.DoubleRow


@functools.lru_cache(maxsize=None)
def build(b=B, d=D, r=R):
    """Build the single-core SPMD program (identical on all cores)."""
    KT = d // 128  # k-tiles over feature dim
    QT = KT // 2  # DoubleRow k-pair tiles
    RT = r // 128  # row tiles per core
    CH = 512  # normalization chunk width
    NCH = b // CH
    LW = min(2048, b)  # DMA piece width
    NLW = b // LW
    GWMAX = 1536 if b >= 1536 else b
    gstart, gwidth = [], []
    pos = 0
    while pos < b:
        w = min(GWMAX, b - pos)
        gstart.append(pos)
        gwidth.append(w)
        pos += w
    NG = len(gstart)

    def group_chunks(g):
        if g >= NG:
            return []
        lo = gstart[g] // CH
        hi = (gstart[g] + gwidth[g]) // CH
        return list(range(lo, hi))

    nc = bacc.Bacc(None, target_bir_lowering=False)
    ftd = nc.dram_tensor("features_t", [d, b], BF16, kind="ExternalInput")
    outd = nc.dram_tensor("losses", [r], F32, kind="ExternalOutput")

    with tile.TileContext(nc) as tc:
        with (
            tc.tile_pool(name="ftp", bufs=1) as ftp,
            tc.tile_pool(name="sing", bufs=1) as sing,
            tc.tile_pool(name="sq", bufs=6) as sqp,
            tc.tile_pool(name="nrm", bufs=3) as nrmp,
            tc.tile_pool(name="dump", bufs=2) as dumpp,
            tc.tile_pool(name="ssq", bufs=2, space="PSUM") as ssqp,
            tc.tile_pool(name="mm", bufs=2, space="PSUM") as mmp,
        ):
            ft = [
                ftp.tile([128, b], BF16, tag=f"ft{k}", name=f"ft{k}")
                for k in range(KT)
            ]
            # fp8 normalized features in DoubleRow layout: dim1 = which
            # k-plane of the pair, so [:, :, cols] slices feed matmul
            # directly as [128, 2, N] APs.
            ft8 = [
                ftp.tile([128, 2, b], FP8, tag=f"ft8_{q}", name=f"ft8_{q}")
                for q in range(QT)
            ]

            # Pre-place the one ACT table set that covers every function
            # used below (exp, ln, copy): natural_log_exp_and_others.
            _ln_exp_set = 6  # index in act_info.json act_func_sets
            _tl = mybir.InstLoadActFuncSet(
                name=nc.get_next_instruction_name(),
                act_func_set_id=_ln_exp_set,
                ins=[],
                outs=[],
            )
            nc.scalar.add_instruction(_tl)

            # ---- loads: sync HWDGE + gpsimd SWDGE, k-tiles 0-1 on sync,
            # 2-3 on gpsimd, piece-major so early chunks land first ----
            for p in range(NLW):
                ls = slice(p * LW, (p + 1) * LW)
                for k in range(KT):
                    eng = nc.sync if k < KT // 2 else nc.gpsimd
                    eng.dma_start(
                        out=ft[k][:, ls],
                        in_=ftd[k * 128 : (k + 1) * 128, ls],
                    )

            # Constant tiles.  negI/eye in bf16 so the diagonal poison can
            # be applied by the PE itself (an accumulating I.T @ (-1e30*I)
            # matmul) instead of a DVE pass over PSUM.
            negI = sing.tile([128, 128], BF16)
            nc.gpsimd.memset(negI[:], 0.0)
            nc.gpsimd.affine_select(
                out=negI[:],
                in_=negI[:],
                compare_op=ALU.not_equal,
                fill=-1e30,
                base=0,
                pattern=[[-1, 128]],
                channel_multiplier=1,
            )
            eyeb = sing.tile([128, 128], BF16)
            nc.gpsimd.memset(eyeb[:], 0.0)
            nc.gpsimd.affine_select(
                out=eyeb[:],
                in_=eyeb[:],
                compare_op=ALU.not_equal,
                fill=1.0,
                base=0,
                pattern=[[-1, 128]],
                channel_multiplier=1,
            )
            # (-1/(FSCALE*T)) * I -- multiplying the partner block by this
            # and row-reducing yields -pos_logit directly.
            negTI = sing.tile([128, 128], F32)
            nc.gpsimd.memset(negTI[:], 0.0)
            nc.gpsimd.affine_select(
                out=negTI[:],
                in_=negTI[:],
                compare_op=ALU.not_equal,
                fill=-INV_T / FSCALE,
                base=0,
                pattern=[[-1, 128]],
                channel_multiplier=1,
            )
            ones_k = sing.tile([128, 1], BF16)
            nc.vector.memset(ones_k[:], 1.0)
            negC = sing.tile([128, 1], F32)
            nc.vector.memset(negC[:], -INV_T)
            ln4c = sing.tile([1, 1], F32)
            nc.vector.memset(ln4c[:], LN_FS_HALF)

            inv_row = sing.tile([1, b], BF16)  # 4/norm per column
            bc_sb = sing.tile([128, b], BF16)  # broadcast 4/norm
            spart = sing.tile([128, RT, NG], F32)
            posn = sing.tile([128, RT], F32)
            ysb = sing.tile([128, RT], F32)
            ssum = sing.tile([128, RT], F32)
            lnS = sing.tile([128, RT], F32)

            # ---- normalization, split so the sum-of-squares half (which
            # gates PE/ACT progress) runs well ahead of the fp8 scale
            # half (needed only one group before use). ----
            def sumsq_chunk(ch):
                cs = slice(ch * CH, (ch + 1) * CH)
                ssq = ssqp.tile([1, CH], F32, name=f"ssq{ch}", tag="ssq")
                for k in range(KT):
                    sq = sqp.tile([128, CH], BF16, name=f"sq{ch}_{k}", tag="sq")
                    nc.vector.tensor_mul(sq[:], ft[k][:, cs], ft[k][:, cs])
                    nc.tensor.matmul(
                        ssq[:],
                        ones_k[:],
                        sq[:],
                        start=(k == 0),
                        stop=(k == KT - 1),
                    )
                lns = nrmp.tile([1, CH], F32, name=f"lns{ch}", tag="lns")
                nc.scalar.activation(out=lns[:], in_=ssq[:], func=AF.Ln)
                nc.scalar.activation(
                    out=inv_row[0:1, cs],
                    in_=lns[:],
                    func=AF.Exp,
                    scale=-0.5,
                    bias=ln4c[:],
                )
                nc.gpsimd.partition_broadcast(bc_sb[:, cs], inv_row[0:1, cs])

            def scale_chunk(ch):
                cs = slice(ch * CH, (ch + 1) * CH)
                for k in range(KT):
                    nc.vector.tensor_mul(
                        ft8[k // 2][:, k % 2, cs], ft[k][:, cs], bc_sb[:, cs]
                    )

            for ch in group_chunks(0) + group_chunks(1):
                sumsq_chunk(ch)
            for ch in group_chunks(0):
                scale_chunk(ch)
            for g in range(NG):
                g0, gw = gstart[g], gwidth[g]
                for ch in group_chunks(g + 1):
                    scale_chunk(ch)
                for t in range(RT):
                    ps = mmp.tile([128, GWMAX], F32, tag="mm")
                    for n2 in range(gw // 512):
                        for q in range(QT):
                            nc.tensor.matmul(
                                ps[:, n2 * 512 : (n2 + 1) * 512],
                                ft8[q][:, :, t * 128 : (t + 1) * 128],
                                ft8[q][:, :, g0 + n2 * 512 : g0 + (n2 + 1) * 512],
                                start=(q == 0),
                                stop=(q == QT - 1),
                                perf_mode=DR,
                            )
                    # Positive-pair diagonal: global col b/2 + t*128.
                    pcol = b // 2 + t * 128
                    if g0 <= pcol < g0 + gw:
                        off = pcol - g0
                        dmp = dumpp.tile([128, 128], F32)
                        nc.vector.tensor_mul(dmp[:], ps[:, off : off + 128], negTI[:])
                        nc.vector.tensor_reduce(
                            out=posn[:, t : t + 1],
                            in_=dmp[:],
                            axis=mybir.AxisListType.X,
                            op=ALU.add,
                        )
                    # Self-similarity diagonal: global col t*128 -> -1e30,
                    # applied on the PE as one extra accumulating matmul.
                    dcol = t * 128
                    if g0 <= dcol < g0 + gw:
                        off = dcol - g0
                        nc.tensor.matmul(
                            ps[:, off : off + 128],
                            eyeb[:],
                            negI[:],
                            start=False,
                            stop=True,
                            skip_group_check=True,
                        )
                    # exp((16cos - 16)/(16T)) in place + fused row-sum.
                    nc.scalar.activation(
                        out=ps[:, :gw],
                        in_=ps[:, :gw],
                        func=AF.Exp,
                        scale=INV_T / FSCALE,
                        bias=negC[:],
                        accum_out=spart[:, t, g : g + 1],
                    )
                # Lookahead sum-of-squares AFTER this group's matmuls, so
                # PE's in-order queue never gates a group on a later DMA.
                for ch in group_chunks(g + 2):
                    sumsq_chunk(ch)

            # ---- epilogue: y = ln(S) - pos/T ----
            for t in range(RT):
                nc.vector.tensor_reduce(
                    out=ssum[:, t : t + 1],
                    in_=spart[:, t, :],
                    axis=mybir.AxisListType.X,
                    op=ALU.add,
                )
            nc.scalar.activation(out=lnS[:, :RT], in_=ssum[:, :RT], func=AF.Ln)
            nc.vector.tensor_add(ysb[:, :RT], lnS[:, :RT], posn[:, :RT])
            nc.sync.dma_start(
                out=outd[:].rearrange("(t p) -> p t", p=128), in_=ysb[:, :RT]
            )

    nc.finalize()
    return nc


def run(features, b=B, d=D, ncores=NCORES, **kwargs):
    """Run the SPMD kernel; returns (losses[b] fp32, BassKernelResults)."""
    r = b // ncores
    nc = build(b, d, r)
    feats = np.ascontiguousarray(np.asarray(features, dtype=np.float32))
    in_maps = []
    for c in range(ncores):
        rot = np.roll(feats, -c * r, axis=0)
        in_maps.append(
            {"features_t": np.ascontiguousarray(rot.T).astype(ml_dtypes.bfloat16)}
        )
    res = run_bass_kernel_spmd(nc, in_maps, core_ids=list(range(ncores)), **kwargs)
    y = np.concatenate([res.results[c]["losses"] for c in range(ncores)])
    return y, res


def kernel(features):
    y, _ = run(features)
    loss = INV_T + float(np.mean(y.astype(np.float64)))
    return np.float32(loss)
